# revision 1
# baseline (speedup 1.0000x reference)
"""2-layer GAT (GATConv x2, PyG-style) on Trainium2, 8 NeuronCores.

Strategy:
  - Nodes sharded by destination across 8 cores (6250 each + pad to 6272).
  - Node ids remapped so each core's shard is contiguous: r(n) = 6272*(n//6250) + n%6250.
  - Per layer, a per-node "table" lives in HBM: 512B rows of 256 fp16 slots:
      [1.0 @0 | feats @1..128 | alpha_src @129 | alpha_dst @130 | garbage].
  - Edge features fetched with dma_gather (Q7 SWDGE): per-edge rows, int16
    indices => table split in lo/hi halves (25088 rows each), edges bucketed.
  - Edges grouped by dst-tile (128 dst nodes), chunks of 128 edges.
    Per chunk: S[e,d] = (iota==dst_local)*ex_e built in ONE DVE op; PE matmul
    lhsT=S, rhs=[ones|feats] accumulates [denom | aggregate] in PSUM.
  - alpha_dst expanded per edge via PE matmul with host-shipped transposed
    one-hots (fp8) x ad vector; ad vector extracted from self-row gathers.
  - Softmax denominator division folded into the tile finalize (per-partition
    scale); exp without max-subtraction (logits are O(1) by construction).
  - Layer-1 output -> new table shard; AllGather collective distributes the
    full table; layer 2 repeats with C=16 output: out2 = (agg @ W2)/denom.
"""

import sys

sys.path.insert(0, "/opt/trn_rl_repo")

import numpy as np

P = 128
SLOTS = 256  # fp16 slots per table row (512B)
TROW_B = 512


# ---------------------------------------------------------------- host prep


def _wrap_idx(v):
    """Flat int array [n] (n % 16 == 0) -> wrapped [128, n//16] int16 layout
    that dma_gather's Q7 cores read (16-partition wrap, replicated x8)."""
    n = v.shape[0]
    w = v.reshape(n // 16, 16).T.astype(np.int16)  # [16, n/16]
    return np.tile(w, (8, 1)).copy()


class Prep:
    """Host-side static preprocessing of the graph for all cores."""

    def __init__(self, n_nodes, n_cores, edge_src, edge_dst):
        self.N = n_nodes
        self.NC = n_cores
        assert n_nodes % n_cores == 0
        self.SHARD = n_nodes // n_cores
        self.SP = ((self.SHARD + P - 1) // P) * P  # padded shard rows
        self.NT = self.SP // P  # dst tiles per core
        self.NPAD = self.SP * n_cores
        self.HALF = self.NPAD // 2
        assert self.HALF % self.SP == 0 and self.HALF < 32768
        self.Z = self.SHARD if self.SHARD < self.SP else 0  # zero row (local)
        # If SHARD == SP there are no trash rows; use row 0 of core 0 shard?
        # That's a real node -> bad for pads. Require SHARD < SP OR handle:
        self.has_trash = self.SHARD < self.SP

        # self loops appended
        src = np.concatenate([edge_src, np.arange(n_nodes, dtype=np.int64)])
        dst = np.concatenate([edge_dst, np.arange(n_nodes, dtype=np.int64)])
        rsrc = (src // self.SHARD) * self.SP + src % self.SHARD
        core = dst // self.SHARD
        sl = dst % self.SHARD
        til = sl // P
        dloc = sl % P
        half = rsrc >= self.HALF
        lidx = rsrc - half * self.HALF  # local idx within half, < HALF

        # bucket edges: [core][tile][half] -> (lidx array, dloc array)
        self.buckets = {}
        order = np.lexsort((dloc, half, til, core))
        src_s, core_s, til_s, half_s, lidx_s, dloc_s = (
            src[order], core[order], til[order], half[order], lidx[order], dloc[order])
        key = ((core_s * self.NT + til_s) * 2 + half_s).astype(np.int64)
        bounds = np.searchsorted(key, np.arange(self.NC * self.NT * 2 + 1))
        for c in range(self.NC):
            for t in range(self.NT):
                for h in range(2):
                    k = (c * self.NT + t) * 2 + h
                    a, b = bounds[k], bounds[k + 1]
                    self.buckets[(c, t, h)] = (lidx_s[a:b].copy(), dloc_s[a:b].copy())

        # inject trash-row denom edges (dst_local valid, idx = Z in lo half)
        if self.has_trash:
            tt = self.NT - 1
            tl = np.arange(self.SHARD % P if self.SHARD % P else P, P)
            for c in range(self.NC):
                li, dl = self.buckets[(c, tt, 0)]
                self.buckets[(c, tt, 0)] = (
                    np.concatenate([li, np.full(len(tl), self.Z, np.int64)]),
                    np.concatenate([dl, tl.astype(np.int64)]),
                )

        # uniform chunk structure across cores
        self.nlo = np.zeros(self.NT, np.int64)
        self.nhi = np.zeros(self.NT, np.int64)
        for t in range(self.NT):
            for c in range(self.NC):
                self.nlo[t] = max(self.nlo[t], -(-len(self.buckets[(c, t, 0)][0]) // P))
                self.nhi[t] = max(self.nhi[t], -(-len(self.buckets[(c, t, 1)][0]) // P))
            self.nlo[t] = max(self.nlo[t], 1)
            self.nhi[t] = max(self.nhi[t], 1)
        self.NCHLO = int(self.nlo.sum())
        self.NCHHI = int(self.nhi.sum())
        self.NCH = self.NCHLO + self.NCHHI
        self.NLOMAX = int(self.nlo.max())
        self.NHIMAX = int(self.nhi.max())
        self.NCHTMAX = int((self.nlo + self.nhi).max())

    def core_arrays(self, c):
        """Build per-core input arrays: idxlo, idxhi, dstl, BT, adl, adh."""
        import ml_dtypes

        ilo, ihi, dstl_cols = [], [], []
        for t in range(self.NT):
            for h in range(2):
                li, dl = self.buckets[(c, t, h)]
                ntarget = (self.nlo if h == 0 else self.nhi)[t] * P
                pad = ntarget - len(li)
                li = np.concatenate([li, np.full(pad, self.Z, np.int64)])
                dl = np.concatenate([dl, np.full(pad, -1, np.int64)])
                (ilo if h == 0 else ihi).append(li)
                dstl_cols.append(dl.reshape(-1, P).T)  # [128, nch]
        # dstl column order: per tile lo chunks then hi chunks
        dstl = np.zeros((P, self.NCH), np.float32)
        col = 0
        klo = 0
        khi = 0
        dcols_lo, dcols_hi = dstl_cols[0::2], dstl_cols[1::2]
        for t in range(self.NT):
            nl, nh = int(self.nlo[t]), int(self.nhi[t])
            dstl[:, col : col + nl] = dcols_lo[t]
            dstl[:, col + nl : col + nl + nh] = dcols_hi[t]
            col += nl + nh
        idxlo = np.concatenate([_wrap_idx(v) for v in ilo], axis=1)
        idxhi = np.concatenate([_wrap_idx(v) for v in ihi], axis=1)

        # BT[d, chunk, e] = 1 where dstl[e, chunk] == d
        bt = np.zeros((P, self.NCH, P), ml_dtypes.float8_e4m3)
        dl = dstl.astype(np.int64)  # [e_part, chunk]
        e_idx, ch_idx = np.meshgrid(np.arange(P), np.arange(self.NCH), indexing="ij")
        valid = dl >= 0
        bt[dl[valid], ch_idx[valid], e_idx[valid]] = 1.0

        # ad-extraction gathers: own shard rows in own half, Z in the other
        own = (c * self.SP + np.arange(self.SP)) % self.HALF
        zv = np.full(self.SP, self.Z, np.int64)
        if c < self.NC // 2:
            adl, adh = _wrap_idx(own), _wrap_idx(zv)
        else:
            adl, adh = _wrap_idx(zv), _wrap_idx(own)
        return idxlo, idxhi, dstl, bt, adl, adh


# ---------------------------------------------------------------- device build


def build_program(pp: Prep, fin, h1, c2, debug=False):
    import os
    STAGE = int(os.environ.get("KSTAGE", "4"))  # 1=dense,2=+adx+edge1,3=+coll,4=full
    KTILES = int(os.environ.get("KTILES", "10000"))
    KPART = int(os.environ.get("KPART", "5"))  # 1=gather,2=+adexp,3=+epre,4=+accum,5=+fin
    import concourse.bass as bass
    import concourse.bacc as bacc
    import concourse.tile as tile
    import concourse.mybir as mybir

    f16, f32, i16, fp8 = (
        mybir.dt.float16, mybir.dt.float32, mybir.dt.int16, mybir.dt.float8e4)
    NT, NPAD, SP = pp.NT, pp.NPAD, pp.SP
    NC = pp.NC

    nc = bacc.Bacc("TRN2", target_bir_lowering=False, debug=debug, num_devices=NC,
                   num_swdge_queues=4)

    # inputs
    xT_d = nc.dram_tensor("xT", [fin, NPAD], f16, kind="ExternalInput")
    w1aug_d = nc.dram_tensor("w1aug", [fin, h1 + 2], f16, kind="ExternalInput")
    w2aug_d = nc.dram_tensor("w2aug", [h1, 2], f16, kind="ExternalInput")
    w2_d = nc.dram_tensor("w2", [h1, c2], f16, kind="ExternalInput")
    b1bc_d = nc.dram_tensor("b1bc", [P, h1], f32, kind="ExternalInput")
    ident_d = nc.dram_tensor("ident", [P, P], f16, kind="ExternalInput")
    iota_d = nc.dram_tensor("iota", [P, P], f16, kind="ExternalInput")
    idxlo_d = nc.dram_tensor("idxlo", [P, pp.NCHLO * 8], i16, kind="ExternalInput")
    idxhi_d = nc.dram_tensor("idxhi", [P, pp.NCHHI * 8], i16, kind="ExternalInput")
    dstl_d = nc.dram_tensor("dstl", [P, pp.NCH], f32, kind="ExternalInput")
    bt_d = nc.dram_tensor("bt", [P, pp.NCH, P], fp8, kind="ExternalInput")
    adl_d = nc.dram_tensor("adl", [P, SP // 16], i16, kind="ExternalInput")
    adh_d = nc.dram_tensor("adh", [P, SP // 16], i16, kind="ExternalInput")
    out_d = nc.dram_tensor("out", [SP, c2], f32, kind="ExternalOutput")

    with tile.TileContext(nc) as tc:
        with (
            tc.tile_pool(name="consts", bufs=1) as cpool,
            tc.tile_pool(name="bigidx", bufs=1) as bigpool,
            tc.tile_pool(name="dense", bufs=3) as dense_pool,
            tc.tile_pool(name="glo", bufs=2) as glo_pool,
            tc.tile_pool(name="adg", bufs=1) as adg_pool,
            tc.tile_pool(name="ghi", bufs=2) as ghi_pool,
            tc.tile_pool(name="bt", bufs=2) as bt_pool,
            tc.tile_pool(name="s", bufs=4) as s_pool,
            tc.tile_pool(name="small", bufs=4) as small_pool,
            tc.tile_pool(name="fin", bufs=2) as fin_pool,
            tc.tile_pool(name="psA", bufs=2, space="PSUM") as psA,  # acc / dense
            tc.tile_pool(name="psB", bufs=2, space="PSUM") as psB,  # adexp
            tc.tile_pool(name="psC", bufs=1, space="PSUM") as psC,  # transpose
            tc.tile_pool(name="psD", bufs=1, space="PSUM") as psD,  # asad / U
            tc.tile_pool(name="dram", bufs=1, space="DRAM") as dram,
        ):
            # ---------------- consts
            w1aug = cpool.tile([fin, h1 + 2], f16)
            nc.sync.dma_start(w1aug[:], w1aug_d[:])
            w2aug = cpool.tile([h1, 2], f16)
            nc.sync.dma_start(w2aug[:], w2aug_d[:])
            w2 = cpool.tile([h1, c2], f16)
            nc.sync.dma_start(w2[:], w2_d[:])
            b1bc = cpool.tile([P, h1], f32)
            nc.sync.dma_start(b1bc[:], b1bc_d[:])
            ident = cpool.tile([P, P], f16)
            nc.sync.dma_start(ident[:], ident_d[:])
            iota = cpool.tile([P, P], f16)
            nc.sync.dma_start(iota[:], iota_d[:])
            idxlo = bigpool.tile([P, pp.NCHLO * 8], i16)
            nc.sync.dma_start(idxlo[:], idxlo_d[:])
            idxhi = bigpool.tile([P, pp.NCHHI * 8], i16)
            nc.sync.dma_start(idxhi[:], idxhi_d[:])
            dstl = bigpool.tile([P, pp.NCH], f32)
            nc.sync.dma_start(dstl[:], dstl_d[:])
            adl_i = bigpool.tile([P, SP // 16], i16)
            nc.sync.dma_start(adl_i[:], adl_d[:])
            adh_i = bigpool.tile([P, SP // 16], i16)
            nc.sync.dma_start(adh_i[:], adh_d[:])

            # tables
            t1 = dram.tile([NPAD, SLOTS], f16)
            t2s = dram.tile([SP, SLOTS], f16)
            t2 = dram.tile([NPAD, SLOTS], f16)

            # ---------------- dense phase: T1 rows = [1 | x@W1 | as | ad]
            for nt in range(NPAD // P):
                xt = dense_pool.tile([fin, P], f16, tag="xt")
                nc.sync.dma_start(xt[:], xT_d[:, nt * P : (nt + 1) * P])
                ps = psA.tile([P, h1 + 2], f32, tag="acc")
                nc.tensor.matmul(ps[:], xt[:], w1aug[:], start=True, stop=True)
                asm = dense_pool.tile([P, SLOTS], f16, tag="asm")
                nc.scalar.copy(asm[:, 1 : h1 + 3], ps[:])
                nc.vector.memset(asm[:, 0:1], 1.0)
                nc.sync.dma_start(t1[nt * P : (nt + 1) * P, :], asm[:])

            def ad_extract(tab, tag):
                """Gather own-shard rows (own half) + zero rows (other half);
                extract the ad column (slot h1+2) summed across halves."""
                ga = adg_pool.tile([P, NT, SLOTS], f16, tag="adg")
                gb = adg_pool.tile([P, NT, SLOTS], f16, tag="adg2")
                for k0 in range(0, NT, 8):
                    k1 = min(k0 + 8, NT)
                    nidx = (k1 - k0) * P
                    nc.gpsimd.dma_gather(
                        ga[:, k0:k1, :], tab[0 : pp.HALF, :],
                        adl_i[:, k0 * 8 : k1 * 8], nidx, nidx, SLOTS,
                        queue_num=(k0 // 8) % 4)
                    nc.gpsimd.dma_gather(
                        gb[:, k0:k1, :], tab[pp.HALF : 2 * pp.HALF, :],
                        adh_i[:, k0 * 8 : k1 * 8], nidx, nidx, SLOTS,
                        queue_num=(k0 // 8 + 2) % 4)
                ad_all = cpool.tile([P, NT], f16, tag=tag)
                nc.vector.tensor_tensor(
                    out=ad_all[:],
                    in0=ga[:, :, h1 + 2],
                    in1=gb[:, :, h1 + 2],
                    op=bass.mybir.AluOpType.add,
                )
                return ad_all

            def edge_layer2(tab, ad_all, layer):
                col = 0
                offlo = 0
                offhi = 0
                for t in range(NT):
                    if t >= KTILES:
                        break
                    nl, nh = int(pp.nlo[t]), int(pp.nhi[t])
                    ncht = nl + nh
                    btt = bt_pool.tile([P, pp.NCHTMAX, P], fp8, tag="btt")
                    nc.sync.dma_start(btt[:, 0:ncht, :], bt_d[:, col : col + ncht, :])
                    glo = glo_pool.tile([P, pp.NLOMAX, SLOTS], f16, tag="glo")
                    for k0 in range(0, nl, 8):
                        k1 = min(k0 + 8, nl)
                        nc.gpsimd.dma_gather(
                            glo[:, k0:k1, :], tab[0 : pp.HALF, :],
                            idxlo[:, (offlo + k0) * 8 : (offlo + k1) * 8],
                            (k1 - k0) * P, (k1 - k0) * P, SLOTS,
                            queue_num=(2 * t + k0 // 8) % 4,
                        )
                    ghi = ghi_pool.tile([P, pp.NHIMAX, SLOTS], f16, tag="ghi")
                    for k0 in range(0, nh, 8):
                        k1 = min(k0 + 8, nh)
                        nc.gpsimd.dma_gather(
                            ghi[:, k0:k1, :], tab[pp.HALF : 2 * pp.HALF, :],
                            idxhi[:, (offhi + k0) * 8 : (offhi + k1) * 8],
                            (k1 - k0) * P, (k1 - k0) * P, SLOTS,
                            queue_num=(2 * t + 1 + k0 // 8) % 4,
                        )
                    if KPART < 2:
                        col += ncht; offlo += nl; offhi += nh
                        continue
                    adexp = psB.tile([P, pp.NCHTMAX], f32, tag="adexp")
                    for j in range(ncht):
                        nc.tensor.matmul(
                            adexp[:, j : j + 1], btt[:, j, :],
                            ad_all[:, t : t + 1], start=True, stop=True,
                        )
                    # epre = as + adexp ; lrelu ; exp
                    if KPART < 3:
                        col += ncht; offlo += nl; offhi += nh
                        continue
                    epre = small_pool.tile([P, pp.NCHTMAX], f32, tag="epre")
                    nc.vector.tensor_tensor(
                        out=epre[:, 0:nl], in0=adexp[:, 0:nl],
                        in1=glo[:, 0:nl, h1 + 1],
                        op=bass.mybir.AluOpType.add,
                    )
                    nc.vector.tensor_tensor(
                        out=epre[:, nl:ncht], in0=adexp[:, nl:ncht],
                        in1=ghi[:, 0:nh, h1 + 1],
                        op=bass.mybir.AluOpType.add,
                    )
                    esc = small_pool.tile([P, pp.NCHTMAX], f32, tag="esc")
                    nc.vector.tensor_scalar_mul(esc[:, 0:ncht], epre[:, 0:ncht], 0.2)
                    nc.vector.tensor_tensor(
                        out=epre[:, 0:ncht], in0=epre[:, 0:ncht],
                        in1=esc[:, 0:ncht], op=bass.mybir.AluOpType.max,
                    )
                    ex = small_pool.tile([P, pp.NCHTMAX], f32, tag="ex")
                    nc.scalar.activation(
                        ex[:, 0:ncht], epre[:, 0:ncht],
                        bass.mybir.ActivationFunctionType.Exp,
                    )
                    # accumulate
                    if KPART < 4:
                        col += ncht; offlo += nl; offhi += nh
                        continue
                    acc = psA.tile([P, h1 + 1], f32, tag="acc")
                    for j in range(ncht):
                        s_t = s_pool.tile([P, P], f16, tag="s")
                        nc.vector.tensor_scalar(
                            out=s_t[:], in0=iota[:],
                            scalar1=dstl[:, col + j : col + j + 1],
                            scalar2=ex[:, j : j + 1],
                            op0=bass.mybir.AluOpType.is_equal,
                            op1=bass.mybir.AluOpType.mult,
                        )
                        g = glo if j < nl else ghi
                        jj = j if j < nl else j - nl
                        nc.tensor.matmul(
                            acc[:], s_t[:], g[:, jj, 0 : h1 + 1],
                            start=(j == 0), stop=(j == ncht - 1),
                        )
                    # finalize
                    if KPART < 5:
                        col += ncht; offlo += nl; offhi += nh
                        continue
                    recip = small_pool.tile([P, 1], f32, tag="recip")
                    nc.vector.reciprocal(recip[:], acc[:, 0:1])
                    if layer == 1:
                        t1f = fin_pool.tile([P, h1], f32, tag="t1f")
                        nc.scalar.activation(
                            t1f[:], acc[:, 1 : h1 + 1],
                            bass.mybir.ActivationFunctionType.Copy,
                            scale=recip[:],
                        )
                        nc.vector.tensor_tensor(
                            out=t1f[:], in0=t1f[:], in1=b1bc[:],
                            op=bass.mybir.AluOpType.add,
                        )
                        asm = fin_pool.tile([P, SLOTS], f16, tag="asm2")
                        nc.vector.tensor_scalar_max(asm[:, 1 : h1 + 1], t1f[:], 0.0)
                        nc.vector.memset(asm[:, 0:1], 1.0)
                        # transpose dance for as2/ad2
                        trp = psC.tile([P, P], f16, tag="trp")
                        nc.tensor.transpose(
                            out=trp[:], in_=asm[:, 1 : h1 + 1], identity=ident[:]
                        )
                        ot = fin_pool.tile([P, P], f16, tag="ot")
                        nc.scalar.copy(ot[:], trp[:])
                        asad = psD.tile([P, 16], f32, tag="asad")
                        nc.tensor.matmul(
                            asad[:, 0:2], ot[:], w2aug[:], start=True, stop=True
                        )
                        nc.vector.tensor_copy(asm[:, h1 + 1 : h1 + 3], asad[:, 0:2])
                        nc.sync.dma_start(t2s[t * P : (t + 1) * P, :], asm[:])
                    else:
                        aggf = fin_pool.tile([P, h1], f16, tag="aggf")
                        nc.scalar.copy(aggf[:], acc[:, 1 : h1 + 1])
                        trp = psC.tile([P, P], f16, tag="trp")
                        nc.tensor.transpose(out=trp[:], in_=aggf[:], identity=ident[:])
                        ot = fin_pool.tile([P, P], f16, tag="ot")
                        nc.scalar.copy(ot[:], trp[:])
                        u = psD.tile([P, 16], f32, tag="asad")
                        nc.tensor.matmul(u[:, 0:c2], ot[:], w2[:], start=True, stop=True)
                        o2 = fin_pool.tile([P, c2], f32, tag="o2")
                        nc.scalar.activation(
                            o2[:], u[:, 0:c2],
                            bass.mybir.ActivationFunctionType.Copy, scale=recip[:],
                        )
                        nc.sync.dma_start(out_d[t * P : (t + 1) * P, :], o2[:])
                    col += ncht
                    offlo += nl
                    offhi += nh

            KADX = int(os.environ.get("KADX", "1"))
            if STAGE >= 2:
                if KADX:
                    ad1 = ad_extract(t1, "ad1")
                else:
                    ad1 = cpool.tile([P, NT], f16, tag="ad1")
                    nc.vector.memset(ad1[:], 0.0)
                edge_layer2(t1, ad1, 1)
            # zero the trash rows of t2s, then exchange
            if STAGE >= 3 and pp.has_trash:
                ztile = cpool.tile([pp.SP - pp.SHARD, SLOTS], f16, tag="zt")
                nc.vector.memset(ztile[:], 0.0)
                nc.sync.dma_start(t2s[pp.SHARD : pp.SP, :], ztile[:])
            if STAGE >= 3:
                nc.gpsimd.collective_compute(
                    "AllGather",
                    bass.mybir.AluOpType.bypass,
                    replica_groups=[list(range(NC))],
                    ins=[t2s.opt()],
                    outs=[t2.opt()],
                )
            if STAGE >= 4:
                ad2 = ad_extract(t2, "ad2")
                edge_layer2(t2, ad2, 2)
            else:
                zo = cpool.tile([P, c2], f32, tag="zo")
                nc.vector.memset(zo[:], 0.0)
                for t in range(NT):
                    nc.sync.dma_start(out_d[t * P : (t + 1) * P, :], zo[:])

    nc.compile()
    return nc


# ---------------------------------------------------------------- entry


def _run(x, edge_index, W1, a_src1, a_dst1, b1, W2, a_src2, a_dst2, b2,
         n_cores=8, trace=False):
    import concourse.mybir as mybir
    from concourse import bass_utils

    N, FIN = x.shape
    H1 = W1.shape[1]
    C2 = W2.shape[1]
    E = edge_index.shape[1]

    pp = Prep(N, n_cores, np.asarray(edge_index[0]), np.asarray(edge_index[1]))
    nc = build_program(pp, FIN, H1, C2)

    # shared input arrays
    xr = np.zeros((pp.NPAD, FIN), np.float32)
    rmap = (np.arange(N) // pp.SHARD) * pp.SP + np.arange(N) % pp.SHARD
    xr[rmap] = x
    xT = xr.T.astype(np.float16).copy()
    w_as1 = W1 @ a_src1
    w_ad1 = W1 @ a_dst1
    w1aug = np.concatenate([W1, w_as1[:, None], w_ad1[:, None]], 1).astype(np.float16)
    w2aug = np.stack([W2 @ a_src2, W2 @ a_dst2], 1).astype(np.float16)
    w2_np = W2.astype(np.float16)
    b1bc = np.broadcast_to(b1.astype(np.float32), (P, H1)).copy()
    ident = np.eye(P, dtype=np.float16)
    iota = np.broadcast_to(np.arange(P, dtype=np.float16), (P, P)).copy()

    in_maps = []
    for c in range(n_cores):
        idxlo, idxhi, dstl, bt, adl, adh = pp.core_arrays(c)
        in_maps.append({
            "xT": xT, "w1aug": w1aug, "w2aug": w2aug, "w2": w2_np,
            "b1bc": b1bc, "ident": ident, "iota": iota,
            "idxlo": idxlo, "idxhi": idxhi, "dstl": dstl, "bt": bt,
            "adl": adl, "adh": adh,
        })

    global _LAST_NC, _LAST_INMAPS
    _LAST_NC, _LAST_INMAPS = nc, in_maps
    res = bass_utils.run_bass_kernel_spmd(
        nc, in_maps, core_ids=list(range(n_cores)), trace=trace
    )
    out = np.concatenate(
        [res.results[c]["out"][: pp.SHARD] for c in range(n_cores)], axis=0
    )
    out = out + b2[None, :].astype(np.float32)
    return out.astype(np.float32), res


def bench_exec(nc, in_maps, n_cores=8, reps=10):
    """Time repeated NEFF executions with device-resident inputs.
    Mirrors bass2jax.run_bass_via_pjrt's multi-core path + timing."""
    import time as _time

    import jax
    import numpy as jnp_np
    from jax.sharding import Mesh, PartitionSpec, NamedSharding
    from jax.experimental.shard_map import shard_map
    import concourse.mybir as mybir
    from concourse import bass2jax

    bass2jax.install_neuronx_cc_hook()
    partition_name = nc.partition_id_tensor.name if nc.partition_id_tensor else None
    in_names, out_names, out_avals, zero_outs = [], [], [], []
    for alloc in nc.m.functions[0].allocations:
        if not isinstance(alloc, mybir.MemoryLocationSet):
            continue
        name = alloc.memorylocations[0].name
        if alloc.kind == "ExternalInput":
            if name != partition_name:
                in_names.append(name)
        elif alloc.kind == "ExternalOutput":
            out_names.append(name)
            shape = tuple(alloc.tensor_shape)
            dtype = mybir.dt.np(alloc.dtype)
            out_avals.append(jax.core.ShapedArray(shape, dtype))
            zero_outs.append(np.zeros(shape, dtype))
    n_params = len(in_names)
    n_outs = len(out_avals)
    in_names.extend(out_names)
    if partition_name is not None:
        in_names.append(partition_name)
    donate = tuple(range(n_params, n_params + n_outs))

    def _body(*args):
        operands = list(args)
        if partition_name is not None:
            operands.append(bass2jax.partition_id_tensor())
        outs = bass2jax._bass_exec_p.bind(
            *operands, out_avals=tuple(out_avals), in_names=tuple(in_names),
            out_names=tuple(out_names), lowering_input_output_aliases=(),
            sim_require_finite=True, sim_require_nnan=True, nc=nc)
        return tuple(outs)

    devices = jax.devices()[:n_cores]
    mesh = Mesh(np.asarray(devices), ("core",))
    sharded = jax.jit(
        shard_map(_body, mesh=mesh,
                  in_specs=(PartitionSpec("core"),) * (n_params + n_outs),
                  out_specs=(PartitionSpec("core"),) * len(out_names),
                  check_rep=False),
        donate_argnums=donate, keep_unused=True)
    sh = NamedSharding(mesh, PartitionSpec("core"))
    concat_in = [
        jax.device_put(
            np.concatenate([np.asarray(in_maps[c][nm]) for c in range(n_cores)], 0), sh)
        for nm in in_names[:n_params]]
    def mkzeros():
        return [jax.device_put(
            np.zeros((n_cores * z.shape[0], *z.shape[1:]), z.dtype), sh)
            for z in zero_outs]
    # warmup
    out = sharded(*concat_in, *mkzeros())
    jax.block_until_ready(out)
    # timed: single calls
    singles = []
    for _ in range(reps):
        zz = mkzeros()
        jax.block_until_ready(zz)
        t0 = _time.perf_counter()
        out = sharded(*concat_in, *zz)
        jax.block_until_ready(out)
        singles.append(_time.perf_counter() - t0)
    # timed: pipelined burst
    zsets = [mkzeros() for _ in range(reps)]
    jax.block_until_ready(zsets)
    t0 = _time.perf_counter()
    outs = [sharded(*concat_in, *z) for z in zsets]
    jax.block_until_ready(outs)
    burst = (_time.perf_counter() - t0) / reps
    return min(singles), burst


def kernel(x, edge_index, W1, a_src1, a_dst1, b1, W2, a_src2, a_dst2, b2):
    out, _ = _run(
        np.asarray(x, np.float32), np.asarray(edge_index),
        np.asarray(W1, np.float32), np.asarray(a_src1, np.float32),
        np.asarray(a_dst1, np.float32), np.asarray(b1, np.float32),
        np.asarray(W2, np.float32), np.asarray(a_src2, np.float32),
        np.asarray(a_dst2, np.float32), np.asarray(b2, np.float32),
    )
    return out



# revision 12
# speedup vs baseline: 1.1945x; 1.1945x over previous
"""2-layer GAT (GATConv x2, PyG-style) on Trainium2, 8 NeuronCores.

Strategy (v2):
  - Nodes degree-balanced across 8 cores and across the 49 dst tiles of each
    core (free permutation; host unpermutes the output).
  - Table layout is collective-chunk-major: local slot l of core c lives at
    global row a = (l//896)*7168 + c*896 + l%896, so the 7 chunked AllGathers
    of layer-1 output write contiguous blocks. 22 reserved (trash) slots per
    core are spread so every core has a zero row in each table half.
  - t1 rows (512B): [1 | x@W1 (128) | as1 | ad1]; t2 rows (256B):
    [1 | h2@W2 (16) | as2 | ad2] - layer-2 aggregates h2@W2 directly, halving
    the collective payload and skipping the layer-2 transpose.
  - Dense phase batched 6 tiles/iter: 1 load, 6 matmuls into packed PSUM,
    2 PSUM->SBUF copies, 1 store (vs 5 instr/tile before).
  - Edges bucketed by (dst tile, src half); within a bucket, per-dst runs are
    bin-packed into 128-edge chunks so each dst lives in exactly ONE chunk
    ("d-disjoint"): ad expansion for ALL chunks of a (tile, half) is a single
    matmul with a union one-hot lhsT and a mask*ad rhs (replaces per-chunk
    1-column matmuls).
  - Gathers merged across 4-tile groups (fewer SWDGE descriptor-gen calls).
  - Per chunk: S[e,d]=(iota==dst)*ex built in one DVE op; PE matmul
    accumulates [denom | payload] in PSUM; softmax division folded into the
    finalize scale.
  - Layer-1 finalize: relu(agg/denom + b1) -> transpose -> one matmul with
    [W2 | W2@a_src2 | W2@a_dst2] gives the whole 19-slot t2 row.
  - AllGather output lives in Shared DRAM; 7 chunk collectives fire as their
    7-tile group finishes, overlapping with layer-1 compute.
"""

import os
import sys

sys.path.insert(0, "/opt/trn_rl_repo")

import numpy as np

P = 128
SLOTS1 = 256  # t1 row: [1 | feats(128) | as | ad] in 256 fp16 slots (512B)
SLOTS2 = 128  # t2 row: [1 | u(16) | as | ad] in 128 fp16 slots (256B)
NCOLL = 7  # collective chunks
CROWS = 896  # rows per collective chunk (per core)
RES = (3, 3, 3, 3, 3, 3, 4)  # reserved trash slots per chunk (sum 22)
TG = 4  # tiles per gather group
DG = 6  # dense tiles per iteration
GCAP = int(os.environ.get("KGCAP", "1024"))  # max idxs per dma_gather call


def _wrap_idx(v):
    """Flat int array [n] (n % 16 == 0) -> wrapped [128, n//16] int16 layout
    that dma_gather's Q7 cores read (16-partition wrap, replicated x8)."""
    n = v.shape[0]
    w = v.reshape(n // 16, 16).T.astype(np.int16)
    return np.tile(w, (8, 1)).copy()


def _ffd(counts, cap=P):
    """First-fit-decreasing bin pack. counts: [n] sizes. Returns (home[n],
    nbins)."""
    order = np.argsort(-counts, kind="stable")
    space = []
    home = np.zeros(len(counts), np.int64)
    for i in order:
        n = counts[i]
        for j in range(len(space)):
            if space[j] >= n:
                space[j] -= n
                home[i] = j
                break
        else:
            home[i] = len(space)
            space.append(cap - n)
    return home, len(space)


class Prep:
    """Host-side static preprocessing of the graph for all cores."""

    def __init__(self, n_nodes, n_cores, edge_src, edge_dst):
        N, NC = n_nodes, n_cores
        self.N, self.NC = N, NC
        assert N % NC == 0
        SHARD = N // NC
        SP = NCOLL * CROWS
        NT = SP // P
        NPAD = SP * NC
        HALF = NPAD // 2
        assert SP - SHARD == sum(RES) and HALF < 32768
        self.SHARD, self.SP, self.NT, self.NPAD, self.HALF = SHARD, SP, NT, NPAD, HALF

        # ---- degree-balanced node -> (core, slot) assignment
        deg = np.bincount(edge_dst, minlength=N) + 1  # + self loop
        order = np.argsort(-deg, kind="stable")
        core_of = np.empty(N, np.int64)
        core_of[order] = np.arange(N) % NC
        resv = np.concatenate(
            [np.arange(i * CROWS + CROWS - RES[i], (i + 1) * CROWS)
             for i in range(NCOLL)])
        free = np.setdiff1d(np.arange(SP), resv)
        free_by_tile = [free[free // P == t] for t in range(NT)]
        cap_t = np.array([len(f) for f in free_by_tile])
        slot_of = np.empty(N, np.int64)
        for c in range(NC):
            nodes_c = order[core_of[order] == c]  # degree-desc
            ptr = np.zeros(NT, np.int64)
            t = 0
            for node in nodes_c:
                while ptr[t % NT] >= cap_t[t % NT]:
                    t += 1
                tt = t % NT
                slot_of[node] = free_by_tile[tt][ptr[tt]]
                ptr[tt] += 1
                t += 1
        self.core_of, self.slot_of = core_of, slot_of

        def addr(c, l):
            return (l // CROWS) * (CROWS * NC) + c * CROWS + (l % CROWS)

        self.addr_of = addr(core_of, slot_of)
        # per-core safe pad rows (reserved slots, zero content) in each half
        self.z_lo = np.array([addr(c, 893) for c in range(NC)])
        self.z_hi = np.array([addr(c, 6 * CROWS + 892) for c in range(NC)])
        assert (self.z_lo < HALF).all() and (self.z_hi >= HALF).all()

        # ---- edges (with self loops) + per-reserved-slot guard edges
        src = np.concatenate([edge_src, np.arange(N, dtype=np.int64)])
        dst = np.concatenate([edge_dst, np.arange(N, dtype=np.int64)])
        e_c = core_of[dst]
        e_slot = slot_of[dst]
        sa = self.addr_of[src]
        e_h = (sa >= HALF).astype(np.int64)
        e_lidx = sa - e_h * HALF
        # guards: one edge per reserved slot per core, in the lo half
        g_c = np.repeat(np.arange(NC), len(resv))
        g_slot = np.tile(resv, NC)
        g_h = np.zeros(len(g_c), np.int64)
        g_lidx = self.z_lo[g_c]
        e_c = np.concatenate([e_c, g_c])
        e_slot = np.concatenate([e_slot, g_slot])
        e_h = np.concatenate([e_h, g_h])
        e_lidx = np.concatenate([e_lidx, g_lidx])
        e_t = e_slot // P
        e_d = e_slot % P

        # ---- bucket sort by (core, tile, half, dst slot)
        so = np.lexsort((e_d, e_h, e_t, e_c))
        e_c, e_t, e_h, e_d, e_lidx = (
            e_c[so], e_t[so], e_h[so], e_d[so], e_lidx[so])
        key = ((e_c * NT + e_t) * 2 + e_h)
        bounds = np.searchsorted(key, np.arange(NC * NT * 2 + 1))

        # ---- d-disjoint chunk packing per bucket
        # chunks[(c,t,h)] -> list of (lidx array, dloc array) per chunk
        self.chunks = {}
        nch = np.zeros((NC, NT, 2), np.int64)
        for c in range(NC):
            for t in range(NT):
                for h in range(2):
                    k = (c * NT + t) * 2 + h
                    a, b = bounds[k], bounds[k + 1]
                    dl, li = e_d[a:b], e_lidx[a:b]
                    dvals, dstart, dcount = np.unique(
                        dl, return_index=True, return_counts=True)
                    assert (dcount <= P).all(), "dst run exceeds one chunk"
                    home, nb = _ffd(dcount)
                    ch_li = [[] for _ in range(nb)]
                    ch_dl = [[] for _ in range(nb)]
                    for ri in range(len(dvals)):
                        j = home[ri]
                        s, n = dstart[ri], dcount[ri]
                        ch_li[j].append(li[s:s + n])
                        ch_dl[j].append(np.full(n, dvals[ri], np.int64))
                    self.chunks[(c, t, h)] = [
                        (np.concatenate(ch_li[j]) if ch_li[j] else
                         np.empty(0, np.int64),
                         np.concatenate(ch_dl[j]) if ch_dl[j] else
                         np.empty(0, np.int64))
                        for j in range(nb)]
                    nch[c, t, h] = nb
        self.NL = nch[:, :, 0].max(axis=0)  # uniform across cores
        self.NH = nch[:, :, 1].max(axis=0)
        self.NCHLO = int(self.NL.sum())
        self.NCHHI = int(self.NH.sum())
        self.NCH = self.NCHLO + self.NCHHI
        self.NLMAX = int(self.NL.max())
        self.NHMAX = int(self.NH.max())
        self.NCHTMAX = int((self.NL + self.NH).max())
        self.colbase = np.concatenate([[0], np.cumsum(self.NL + self.NH)[:-1]])

        # ---- gather groups of TG tiles
        self.GROUPS = [list(range(t0, min(t0 + TG, NT)))
                       for t0 in range(0, NT, TG)]
        self.g_lo = [int(sum(self.NL[t] for t in g)) for g in self.GROUPS]
        self.g_hi = [int(sum(self.NH[t] for t in g)) for g in self.GROUPS]
        self.GLOMAX = max(self.g_lo)
        self.GHIMAX = max(self.g_hi)
        self.ic_lo = np.concatenate([[0], np.cumsum(self.g_lo)[:-1]])
        self.ic_hi = np.concatenate([[0], np.cumsum(self.g_hi)[:-1]])

    def core_arrays(self, c):
        """Per-core inputs: idxlo, idxhi, dstl, admask, btall, adl, adh."""
        NT, HALF, SP = self.NT, self.HALF, self.SP
        zlo = self.z_lo[c] - 0
        zhi = self.z_hi[c] - HALF
        dstl = np.full((P, self.NCH), -1.0, np.float32)
        admask = np.zeros((P, self.NCH), np.float16)
        btall = np.zeros((P, NT * 2 * P), np.float16)
        ilo, ihi = [], []
        for g in self.GROUPS:
            for h in (0, 1):
                for t in g:
                    chl = self.chunks[(c, t, h)]
                    ntarget = int((self.NL if h == 0 else self.NH)[t])
                    col0 = int(self.colbase[t]) + (int(self.NL[t]) if h else 0)
                    for j in range(ntarget):
                        li, dl = (chl[j] if j < len(chl)
                                  else (np.empty(0, np.int64),) * 2)
                        pad = P - len(li)
                        li = np.concatenate(
                            [li, np.full(pad, zlo if h == 0 else zhi)])
                        (ilo if h == 0 else ihi).append(li)
                        dstl[:len(dl), col0 + j] = dl
                        e_pos = np.arange(len(dl))
                        btall[dl, (t * 2 + h) * P + e_pos] = 1.0
                        if len(dl):
                            runs = np.unique(dl)
                            admask[runs, col0 + j] = 1.0
        idxlo = _wrap_idx(np.concatenate(ilo)) if ilo else np.zeros(
            (P, 0), np.int16)
        idxhi = _wrap_idx(np.concatenate(ihi)) if ihi else np.zeros(
            (P, 0), np.int16)

        # ad-extract gathers: own-shard rows in their half, safe row in other
        own = np.array([(l // CROWS) * (CROWS * self.NC) + c * CROWS
                        + (l % CROWS) for l in range(SP)])
        lo = np.where(own < HALF, own, self.z_lo[c])
        hi = np.where(own >= HALF, own - HALF, zhi)
        return (idxlo, idxhi, dstl, admask, btall,
                _wrap_idx(lo), _wrap_idx(hi))


# ---------------------------------------------------------------- device build


def build_program(pp: Prep, fin, h1, c2, debug=False):
    STAGE = int(os.environ.get("KSTAGE", "4"))
    KCOLL = int(os.environ.get("KCOLL", "7"))  # 7=chunked Local, 1=one Shared
    import concourse.bass as bass
    import concourse.bacc as bacc
    import concourse.tile as tile
    import concourse.mybir as mybir

    f16, f32, i16 = mybir.dt.float16, mybir.dt.float32, mybir.dt.int16
    NT, NPAD, SP, HALF = pp.NT, pp.NPAD, pp.SP, pp.HALF
    NC = pp.NC
    AS1, AD1 = h1 + 1, h1 + 2  # t1 slots: 129, 130
    AS2, AD2 = c2 + 1, c2 + 2  # t2 slots: 17, 18
    RHS1, RHS2 = h1 + 1, c2 + 1  # accum rhs widths: 129, 17
    NDT = NPAD // P  # 392 dense tiles

    nc = bacc.Bacc("TRN2", target_bir_lowering=False, debug=debug,
                   num_devices=NC, num_swdge_queues=4)

    xT_d = nc.dram_tensor("xT", [fin, NPAD], f16, kind="ExternalInput")
    w1aug_d = nc.dram_tensor("w1aug", [fin, h1 + 2], f16, kind="ExternalInput")
    w2cat_d = nc.dram_tensor("w2cat", [h1, c2 + 2], f16, kind="ExternalInput")
    b1bc_d = nc.dram_tensor("b1bc", [P, h1], f32, kind="ExternalInput")
    ident_d = nc.dram_tensor("ident", [P, P], f16, kind="ExternalInput")
    iota_d = nc.dram_tensor("iota", [P, P], f16, kind="ExternalInput")
    idxlo_d = nc.dram_tensor("idxlo", [P, pp.NCHLO * 8], i16,
                             kind="ExternalInput")
    idxhi_d = nc.dram_tensor("idxhi", [P, pp.NCHHI * 8], i16,
                             kind="ExternalInput")
    dstl_d = nc.dram_tensor("dstl", [P, pp.NCH], f32, kind="ExternalInput")
    admask_d = nc.dram_tensor("admask", [P, pp.NCH], f16, kind="ExternalInput")
    btall_d = nc.dram_tensor("btall", [P, NT * 2 * P], f16,
                             kind="ExternalInput")
    adl_d = nc.dram_tensor("adl", [P, SP // 16], i16, kind="ExternalInput")
    adh_d = nc.dram_tensor("adh", [P, SP // 16], i16, kind="ExternalInput")
    out_d = nc.dram_tensor("out", [SP, c2], f32, kind="ExternalOutput")

    with tile.TileContext(nc) as tc:
        with (
            tc.tile_pool(name="consts", bufs=1) as cpool,
            tc.tile_pool(name="bigidx", bufs=1) as bigpool,
            tc.tile_pool(name="dense", bufs=2) as dense_pool,
            tc.tile_pool(name="glo", bufs=2) as glo_pool,
            tc.tile_pool(name="ghi", bufs=2) as ghi_pool,
            tc.tile_pool(name="adg", bufs=2) as adg_pool,
            tc.tile_pool(name="bt", bufs=2) as bt_pool,
            tc.tile_pool(name="s", bufs=4) as s_pool,
            tc.tile_pool(name="small", bufs=4) as small_pool,
            tc.tile_pool(name="fin", bufs=2) as fin_pool,
            tc.tile_pool(name="psA", bufs=2, space="PSUM") as psA,
            tc.tile_pool(name="psB", bufs=2, space="PSUM") as psB,
            tc.tile_pool(name="psC", bufs=1, space="PSUM") as psC,
            tc.tile_pool(name="psD", bufs=1, space="PSUM") as psD,
            tc.tile_pool(name="dram", bufs=1, space="DRAM") as dram,
        ):
            # ---------------- consts
            w1aug = cpool.tile([fin, h1 + 2], f16)
            nc.sync.dma_start(w1aug[:], w1aug_d[:])
            w2cat = cpool.tile([h1, c2 + 2], f16)
            nc.sync.dma_start(w2cat[:], w2cat_d[:])
            b1bc = cpool.tile([P, h1], f32)
            nc.sync.dma_start(b1bc[:], b1bc_d[:])
            ident = cpool.tile([P, P], f16)
            nc.sync.dma_start(ident[:], ident_d[:])
            iota = cpool.tile([P, P], f16)
            nc.sync.dma_start(iota[:], iota_d[:])
            idxlo = bigpool.tile([P, pp.NCHLO * 8], i16)
            nc.sync.dma_start(idxlo[:], idxlo_d[:])
            idxhi = bigpool.tile([P, pp.NCHHI * 8], i16)
            nc.sync.dma_start(idxhi[:], idxhi_d[:])
            dstl = bigpool.tile([P, pp.NCH], f32)
            nc.sync.dma_start(dstl[:], dstl_d[:])
            admask = bigpool.tile([P, pp.NCH], f16)
            nc.sync.dma_start(admask[:], admask_d[:])
            adl_i = bigpool.tile([P, SP // 16], i16)
            nc.sync.dma_start(adl_i[:], adl_d[:])
            adh_i = bigpool.tile([P, SP // 16], i16)
            nc.sync.dma_start(adh_i[:], adh_d[:])
            ztile = cpool.tile([4, SLOTS2], f16)
            nc.vector.memset(ztile[:], 0.0)

            t1 = dram.tile([NPAD, SLOTS1], f16)
            t2s = dram.tile([SP, SLOTS2], f16)
            t2 = dram.tile([NPAD, SLOTS2], f16)

            # ---------------- dense phase: t1 rows = [1 | x@W1 | as | ad]
            n_dense = NDT if STAGE >= 1 else 0
            for g0 in range(0, n_dense, DG):
                n_t = min(DG, NDT - g0)
                xt = dense_pool.tile([fin, DG * P], f16, tag="xt")
                nc.sync.dma_start(
                    xt[:, 0:n_t * P], xT_d[:, g0 * P:(g0 + n_t) * P])
                asm = dense_pool.tile([P, DG, SLOTS1], f16, tag="asm")
                for b in range((n_t + 2) // 3):
                    nb = min(3, n_t - 3 * b)
                    ps = psA.tile([P, 3 * (h1 + 2)], f32, tag=f"dense{b}",
                                  bufs=1)
                    for jj in range(nb):
                        j = 3 * b + jj
                        nc.tensor.matmul(
                            ps[:, jj * (h1 + 2):(jj + 1) * (h1 + 2)],
                            xt[:, j * P:(j + 1) * P], w1aug[:],
                            start=True, stop=True)
                    nc.scalar.copy(
                        asm[:, 3 * b:3 * b + nb, 1:h1 + 3],
                        ps[:, 0:nb * (h1 + 2)].rearrange(
                            "p (j s) -> p j s", s=h1 + 2))
                nc.vector.memset(asm[:, 0:n_t, 0:1], 1.0)
                nc.sync.dma_start(
                    t1[g0 * P:(g0 + n_t) * P, :].rearrange(
                        "(j p) s -> p j s", p=P),
                    asm[:, 0:n_t, :])

            # ---------------- ad1 extraction (7 rounds of paired gathers)
            ad1 = cpool.tile([P, NT], f32, tag="ad1")
            if STAGE >= 2:
                for k in range(NCOLL):
                    ga = adg_pool.tile([P, NCOLL, SLOTS1], f16, tag="ga")
                    gb = adg_pool.tile([P, NCOLL, SLOTS1], f16, tag="gb")
                    nc.gpsimd.dma_gather(
                        ga[:], t1[0:HALF, :], adl_i[:, k * 56:(k + 1) * 56],
                        CROWS, CROWS, SLOTS1, queue_num=(2 * k) % 4)
                    nc.gpsimd.dma_gather(
                        gb[:], t1[HALF:NPAD, :], adh_i[:, k * 56:(k + 1) * 56],
                        CROWS, CROWS, SLOTS1, queue_num=(2 * k + 1) % 4)
                    nc.vector.tensor_tensor(
                        out=ad1[:, k * NCOLL:(k + 1) * NCOLL],
                        in0=ga[:, :, AD1], in1=gb[:, :, AD1],
                        op=bass.mybir.AluOpType.add)

            def edge_layer(tab, ad_all, layer):
                slots = SLOTS1 if layer == 1 else SLOTS2
                as_slot = AS1 if layer == 1 else AS2
                rhs_w = RHS1 if layer == 1 else RHS2
                for g, tiles in enumerate(pp.GROUPS):
                    nlo_g, nhi_g = pp.g_lo[g], pp.g_hi[g]
                    gloF = glo_pool.tile([P, pp.GLOMAX * SLOTS1], f16,
                                         tag="glo")
                    ghiF = ghi_pool.tile([P, pp.GHIMAX * SLOTS1], f16,
                                         tag="ghi")
                    glo = gloF[:].rearrange("p (n s) -> p n s", s=slots)
                    ghi = ghiF[:].rearrange("p (n s) -> p n s", s=slots)
                    cap = GCAP // P
                    for s0 in range(0, nlo_g, cap):
                        s1 = min(s0 + cap, nlo_g)
                        ic = (int(pp.ic_lo[g]) + s0) * 8
                        nc.gpsimd.dma_gather(
                            glo[:, s0:s1, :], tab[0:HALF, :],
                            idxlo[:, ic:ic + (s1 - s0) * 8],
                            (s1 - s0) * P, (s1 - s0) * P, slots,
                            queue_num=(2 * g) % 4)
                    for s0 in range(0, nhi_g, cap):
                        s1 = min(s0 + cap, nhi_g)
                        ic = (int(pp.ic_hi[g]) + s0) * 8
                        nc.gpsimd.dma_gather(
                            ghi[:, s0:s1, :], tab[HALF:NPAD, :],
                            idxhi[:, ic:ic + (s1 - s0) * 8],
                            (s1 - s0) * P, (s1 - s0) * P, slots,
                            queue_num=(2 * g + 1) % 4)
                    btg = bt_pool.tile([P, TG * 2 * P], f16, tag="btg")
                    nc.sync.dma_start(
                        btg[:, 0:len(tiles) * 2 * P],
                        btall_d[:, tiles[0] * 2 * P:
                                (tiles[-1] + 1) * 2 * P])
                    loff = 0
                    hoff = 0
                    for ti, t in enumerate(tiles):
                        nl, nh = int(pp.NL[t]), int(pp.NH[t])
                        ncht = nl + nh
                        col = int(pp.colbase[t])
                        # ad expansion: one matmul per half
                        adexp = psB.tile([P, pp.NCHTMAX], f32, tag="adexp")
                        adrep = small_pool.tile([P, pp.NCHTMAX], f16,
                                                tag="adrep")
                        nc.vector.tensor_scalar_mul(
                            adrep[:, 0:ncht], admask[:, col:col + ncht],
                            ad_all[:, t:t + 1])
                        if nl:
                            nc.tensor.matmul(
                                adexp[:, 0:nl],
                                btg[:, (ti * 2) * P:(ti * 2 + 1) * P],
                                adrep[:, 0:nl], start=True, stop=True)
                        if nh:
                            nc.tensor.matmul(
                                adexp[:, nl:ncht],
                                btg[:, (ti * 2 + 1) * P:(ti * 2 + 2) * P],
                                adrep[:, nl:ncht], start=True, stop=True)
                        # epre = as + adexp ; lrelu ; exp
                        epre = small_pool.tile([P, pp.NCHTMAX], f32,
                                               tag="epre")
                        nc.vector.tensor_tensor(
                            out=epre[:, 0:nl], in0=adexp[:, 0:nl],
                            in1=glo[:, loff:loff + nl, as_slot],
                            op=bass.mybir.AluOpType.add)
                        nc.vector.tensor_tensor(
                            out=epre[:, nl:ncht], in0=adexp[:, nl:ncht],
                            in1=ghi[:, hoff:hoff + nh, as_slot],
                            op=bass.mybir.AluOpType.add)
                        esc = small_pool.tile([P, pp.NCHTMAX], f32, tag="esc")
                        nc.vector.tensor_scalar_mul(
                            esc[:, 0:ncht], epre[:, 0:ncht], 0.2)
                        nc.vector.tensor_tensor(
                            out=epre[:, 0:ncht], in0=epre[:, 0:ncht],
                            in1=esc[:, 0:ncht], op=bass.mybir.AluOpType.max)
                        ex = small_pool.tile([P, pp.NCHTMAX], f32, tag="ex")
                        nc.scalar.activation(
                            ex[:, 0:ncht], epre[:, 0:ncht],
                            bass.mybir.ActivationFunctionType.Exp)
                        # accumulate [denom | payload]
                        acc = psA.tile([P, RHS1], f32, tag="acc")
                        for j in range(ncht):
                            s_t = s_pool.tile([P, P], f16, tag="s")
                            nc.vector.tensor_scalar(
                                out=s_t[:], in0=iota[:],
                                scalar1=dstl[:, col + j:col + j + 1],
                                scalar2=ex[:, j:j + 1],
                                op0=bass.mybir.AluOpType.is_equal,
                                op1=bass.mybir.AluOpType.mult)
                            g_t = glo if j < nl else ghi
                            jj = loff + j if j < nl else hoff + j - nl
                            nc.tensor.matmul(
                                acc[:, 0:rhs_w], s_t[:],
                                g_t[:, jj, 0:rhs_w],
                                start=(j == 0), stop=(j == ncht - 1))
                        # finalize
                        i_coll = t // NCOLL
                        jj7 = t % NCOLL
                        recip = small_pool.tile([P, 1], f32, tag="recip")
                        nc.vector.reciprocal(recip[:], acc[:, 0:1])
                        if layer == 1:
                            if jj7 == 0:
                                asmG = fin_pool.tile([P, NCOLL, SLOTS2], f16,
                                                     tag="asmG")
                            t1f = fin_pool.tile([P, h1], f32, tag="t1f")
                            nc.scalar.activation(
                                t1f[:], acc[:, 1:h1 + 1],
                                bass.mybir.ActivationFunctionType.Copy,
                                scale=recip[:])
                            nc.vector.tensor_tensor(
                                out=t1f[:], in0=t1f[:], in1=b1bc[:],
                                op=bass.mybir.AluOpType.add)
                            h2sb = fin_pool.tile([P, h1], f16, tag="h2sb")
                            nc.vector.tensor_scalar_max(h2sb[:], t1f[:], 0.0)
                            trp = psC.tile([P, P], f16, tag="trp")
                            nc.tensor.transpose(
                                out=trp[:], in_=h2sb[:], identity=ident[:])
                            ot = fin_pool.tile([P, P], f16, tag="ot")
                            nc.scalar.copy(ot[:], trp[:])
                            uv = psD.tile([P, c2 + 2], f32, tag="uv")
                            nc.tensor.matmul(uv[:], ot[:], w2cat[:],
                                             start=True, stop=True)
                            nc.vector.tensor_copy(
                                asmG[:, jj7, 1:c2 + 3], uv[:])
                            nc.vector.memset(asmG[:, jj7, 0:1], 1.0)
                            if jj7 == NCOLL - 1:
                                r0 = i_coll * CROWS
                                nc.sync.dma_start(
                                    t2s[r0:r0 + CROWS, :].rearrange(
                                        "(j p) s -> p j s", p=P),
                                    asmG[:])
                                nres = RES[i_coll]
                                nc.sync.dma_start(
                                    t2s[r0 + CROWS - nres:r0 + CROWS, :],
                                    ztile[0:nres, :])
                                if STAGE >= 3 and KCOLL == 7:
                                    nc.gpsimd.collective_compute(
                                        "AllGather",
                                        bass.mybir.AluOpType.bypass,
                                        replica_groups=[list(range(NC))],
                                        ins=[t2s[r0:r0 + CROWS, :]],
                                        outs=[t2[r0 * NC:(r0 + CROWS) * NC,
                                                 :]],
                                    )
                        else:
                            if jj7 == 0:
                                o2G = fin_pool.tile([P, NCOLL, c2], f32,
                                                    tag="o2G")
                            nc.scalar.activation(
                                o2G[:, jj7, :], acc[:, 1:c2 + 1],
                                bass.mybir.ActivationFunctionType.Copy,
                                scale=recip[:])
                            if jj7 == NCOLL - 1:
                                r0 = i_coll * CROWS
                                nc.sync.dma_start(
                                    out_d[r0:r0 + CROWS, :].rearrange(
                                        "(j p) s -> p j s", p=P),
                                    o2G[:])
                        loff += nl
                        hoff += nh

            if STAGE >= 2:
                edge_layer(t1, ad1, 1)
            if STAGE >= 3 and KCOLL == 1:
                # single Shared-output AllGather (rank-major), then one local
                # strided DMA permutes into the chunk-major t2 layout
                t2rm = dram.tile([NPAD, SLOTS2], f16, addr_space="Shared")
                nc.gpsimd.collective_compute(
                    "AllGather", bass.mybir.AluOpType.bypass,
                    replica_groups=[list(range(NC))],
                    ins=[t2s[:]], outs=[t2rm[:]])
                nc.sync.dma_start(
                    t2[:].rearrange("(i c m) s -> c i m s", c=NC, m=CROWS),
                    t2rm[:].rearrange("(c i m) s -> c i m s", i=NCOLL,
                                      m=CROWS))
            if STAGE >= 4:
                # ad2: strided column read of t2s (local; overlaps collectives)
                ad2h = cpool.tile([P, NT], f16, tag="ad2h")
                nc.sync.dma_start(
                    ad2h[:],
                    t2s[:].rearrange("(t p) s -> p t s", p=P)[:, :, AD2])
                ad2 = cpool.tile([P, NT], f32, tag="ad2")
                nc.vector.tensor_copy(ad2[:], ad2h[:])
                edge_layer(t2, ad2, 2)
            else:
                zo = cpool.tile([P, NCOLL, c2], f32, tag="zo")
                nc.vector.memset(zo[:], 0.0)
                for i in range(NCOLL):
                    nc.sync.dma_start(
                        out_d[i * CROWS:(i + 1) * CROWS, :].rearrange(
                            "(j p) s -> p j s", p=P),
                        zo[:])

    nc.compile()
    return nc


# ---------------------------------------------------------------- entry


def _run(x, edge_index, W1, a_src1, a_dst1, b1, W2, a_src2, a_dst2, b2,
         n_cores=8, trace=False):
    from concourse import bass_utils

    N, FIN = x.shape
    H1 = W1.shape[1]
    C2 = W2.shape[1]

    pp = Prep(N, n_cores, np.asarray(edge_index[0]), np.asarray(edge_index[1]))
    nc = build_program(pp, FIN, H1, C2)

    xr = np.zeros((pp.NPAD, FIN), np.float32)
    xr[pp.addr_of] = x
    xT = xr.T.astype(np.float16).copy()
    w1aug = np.concatenate(
        [W1, (W1 @ a_src1)[:, None], (W1 @ a_dst1)[:, None]], 1
    ).astype(np.float16)
    w2cat = np.concatenate(
        [W2, (W2 @ a_src2)[:, None], (W2 @ a_dst2)[:, None]], 1
    ).astype(np.float16)
    b1bc = np.broadcast_to(b1.astype(np.float32), (P, H1)).copy()
    ident = np.eye(P, dtype=np.float16)
    iota = np.broadcast_to(np.arange(P, dtype=np.float16), (P, P)).copy()

    in_maps = []
    for c in range(n_cores):
        idxlo, idxhi, dstl, admask, btall, adl, adh = pp.core_arrays(c)
        in_maps.append({
            "xT": xT, "w1aug": w1aug, "w2cat": w2cat, "b1bc": b1bc,
            "ident": ident, "iota": iota, "idxlo": idxlo, "idxhi": idxhi,
            "dstl": dstl, "admask": admask, "btall": btall,
            "adl": adl, "adh": adh,
        })

    global _LAST_NC, _LAST_INMAPS
    _LAST_NC, _LAST_INMAPS = nc, in_maps
    res = bass_utils.run_bass_kernel_spmd(
        nc, in_maps, core_ids=list(range(n_cores)), trace=trace
    )
    out = np.empty((N, C2), np.float32)
    for c in range(n_cores):
        sel = pp.core_of == c
        out[sel] = res.results[c]["out"][pp.slot_of[sel]]
    out = out + b2[None, :].astype(np.float32)
    return out.astype(np.float32), res


def bench_exec(nc, in_maps, n_cores=8, reps=10):
    """Time repeated NEFF executions with device-resident inputs."""
    import time as _time

    import jax
    from jax.sharding import Mesh, PartitionSpec, NamedSharding
    from jax.experimental.shard_map import shard_map
    import concourse.mybir as mybir
    from concourse import bass2jax

    bass2jax.install_neuronx_cc_hook()
    partition_name = nc.partition_id_tensor.name if nc.partition_id_tensor else None
    in_names, out_names, out_avals, zero_outs = [], [], [], []
    for alloc in nc.m.functions[0].allocations:
        if not isinstance(alloc, mybir.MemoryLocationSet):
            continue
        name = alloc.memorylocations[0].name
        if alloc.kind == "ExternalInput":
            if name != partition_name:
                in_names.append(name)
        elif alloc.kind == "ExternalOutput":
            out_names.append(name)
            shape = tuple(alloc.tensor_shape)
            dtype = mybir.dt.np(alloc.dtype)
            out_avals.append(jax.core.ShapedArray(shape, dtype))
            zero_outs.append(np.zeros(shape, dtype))
    n_params = len(in_names)
    n_outs = len(out_avals)
    in_names.extend(out_names)
    if partition_name is not None:
        in_names.append(partition_name)
    donate = tuple(range(n_params, n_params + n_outs))

    def _body(*args):
        operands = list(args)
        if partition_name is not None:
            operands.append(bass2jax.partition_id_tensor())
        outs = bass2jax._bass_exec_p.bind(
            *operands, out_avals=tuple(out_avals), in_names=tuple(in_names),
            out_names=tuple(out_names), lowering_input_output_aliases=(),
            sim_require_finite=True, sim_require_nnan=True, nc=nc)
        return tuple(outs)

    devices = jax.devices()[:n_cores]
    mesh = Mesh(np.asarray(devices), ("core",))
    sharded = jax.jit(
        shard_map(_body, mesh=mesh,
                  in_specs=(PartitionSpec("core"),) * (n_params + n_outs),
                  out_specs=(PartitionSpec("core"),) * len(out_names),
                  check_rep=False),
        donate_argnums=donate, keep_unused=True)
    sh = NamedSharding(mesh, PartitionSpec("core"))
    concat_in = [
        jax.device_put(
            np.concatenate([np.asarray(in_maps[c][nm]) for c in range(n_cores)], 0), sh)
        for nm in in_names[:n_params]]
    def mkzeros():
        return [jax.device_put(
            np.zeros((n_cores * z.shape[0], *z.shape[1:]), z.dtype), sh)
            for z in zero_outs]
    out = sharded(*concat_in, *mkzeros())
    jax.block_until_ready(out)
    singles = []
    for _ in range(reps):
        zz = mkzeros()
        jax.block_until_ready(zz)
        t0 = _time.perf_counter()
        out = sharded(*concat_in, *zz)
        jax.block_until_ready(out)
        singles.append(_time.perf_counter() - t0)
    zsets = [mkzeros() for _ in range(reps)]
    jax.block_until_ready(zsets)
    t0 = _time.perf_counter()
    outs = [sharded(*concat_in, *z) for z in zsets]
    jax.block_until_ready(outs)
    burst = (_time.perf_counter() - t0) / reps
    return min(singles), burst


def kernel(x, edge_index, W1, a_src1, a_dst1, b1, W2, a_src2, a_dst2, b2):
    out, _ = _run(
        np.asarray(x, np.float32), np.asarray(edge_index),
        np.asarray(W1, np.float32), np.asarray(a_src1, np.float32),
        np.asarray(a_dst1, np.float32), np.asarray(b1, np.float32),
        np.asarray(W2, np.float32), np.asarray(a_src2, np.float32),
        np.asarray(a_dst2, np.float32), np.asarray(b2, np.float32),
    )
    return out
